# revision 8
# baseline (speedup 1.0000x reference)
"""Trainium2 Bass kernel for nn_CondensedEmbracementLayer.

out[b, j] = tokens[b, idx[b, j], j] where
  idx[b, j] = min(int(u[b, j] * L_b), SEQ-1),
  L_b = max(count_leading_ones(mask[b]) - 1, 1),
  u = jax.random.uniform(key=42, (BS, EMB)) -- input-independent constant.

Sharding: pure data parallel, 8 cores x 8 batches.

Host computes idx by mirroring the reference's jax ops on the default
backend (bit-exact with the reference on this platform) and builds per-batch
int16 chunk-gather tables. Each core's Bass kernel, per batch:
  - dma_gather's 1024 256B chunks (the chunk containing the sampled element
    of each output column; 1024 idxs is the SWDGE ring cap per call),
  - extracts the target element per chunk via mask-multiply + 64-wide
    reduce on DVE (the target's position within its chunk is p%64, a
    compile-time constant mask),
then transposes the [128, 64] result on PE so the store is one DMA of 64
contiguous 512B lines.
"""

import numpy as np

BS, SEQ, EMB = 64, 512, 1024
NCORES = 8
BPC = BS // NCORES         # batches per core
NI = EMB                   # 1024 gather indices per call (one batch)
COLS = NI // 16            # 64 idx-table cols (16-partition wrap)
ELEM = 64                  # f32 per gathered chunk (256B)
SLOTS = NI // 128          # 8 dst slots per partition

_STATE = {}


def _get_module():
    if "nc" in _STATE:
        return _STATE["nc"]
    import concourse.bacc as bacc
    import concourse.mybir as mybir
    import concourse.tile as tile
    from concourse.masks import make_identity

    nc = bacc.Bacc("TRN2", target_bir_lowering=False, debug=False,
                   num_swdge_queues=4)
    tok = nc.dram_tensor("tok", [BPC, SEQ, EMB], mybir.dt.float32, kind="ExternalInput")
    gidx = nc.dram_tensor("gidx", [128, BPC * COLS], mybir.dt.int16, kind="ExternalInput")
    mconst = nc.dram_tensor("mconst", [128, ELEM], mybir.dt.float32, kind="ExternalInput")
    out = nc.dram_tensor("out", [BPC, EMB], mybir.dt.float32, kind="ExternalOutput")

    with tile.TileContext(nc) as tc:
        with (
            tc.tile_pool(name="sb", bufs=1) as pool,
            tc.tile_pool(name="ps", bufs=1, space="PSUM") as psum,
        ):
            ident = pool.tile([128, 128], mybir.dt.float32)
            make_identity(nc, ident[:])
            msk = pool.tile([128, ELEM], mybir.dt.float32)
            nc.sync.dma_start(msk[:], mconst.ap())
            # all idx tables in one load (BPC*COLS i16 per partition = 1 KiB)
            idx_sb = pool.tile([128, BPC * COLS], mybir.dt.int16)
            nc.sync.dma_start(idx_sb[:], gidx.ap())

            val_all = pool.tile([128, 64], mybir.dt.float32)
            for b in range(BPC):
                dst = pool.tile([128, SLOTS * ELEM], mybir.dt.float32, tag=f"dst{b % 4}")
                table = (
                    tok.ap()[b]
                    .rearrange("s e -> (s e)")
                    .rearrange("(r c) -> r c", c=ELEM)
                )
                nc.gpsimd.dma_gather(
                    dst[:].rearrange("p (q e) -> p q e", e=ELEM),
                    table,
                    idx_sb[:, b * COLS:(b + 1) * COLS],
                    NI,
                    NI,
                    ELEM,
                    queue_num=b % 4,
                )
                masked = pool.tile([128, SLOTS * ELEM], mybir.dt.float32, tag=f"m{b % 4}")
                nc.vector.tensor_tensor(
                    out=masked[:].rearrange("p (q e) -> p q e", e=ELEM),
                    in0=dst[:].rearrange("p (q e) -> p q e", e=ELEM),
                    in1=msk[:].unsqueeze(1).to_broadcast([128, SLOTS, ELEM]),
                    op=mybir.AluOpType.mult,
                )
                nc.vector.reduce_sum(
                    out=val_all[:, b * SLOTS:(b + 1) * SLOTS],
                    in_=masked[:].rearrange("p (q e) -> p q e", e=ELEM),
                    axis=mybir.AxisListType.X,
                )
            tpsum = psum.tile([64, 128], mybir.dt.float32, space="PSUM")
            nc.tensor.transpose(out=tpsum[:], in_=val_all[:], identity=ident[:])
            tsb = pool.tile([64, 128], mybir.dt.float32)
            nc.vector.tensor_copy(tsb[:], tpsum[:])
            dram_view = out.ap().rearrange("b (q i) -> (b q) i", i=128)
            nc.sync.dma_start(dram_view, tsb[:])
    nc.compile()
    _STATE["nc"] = nc
    return nc


def _indices(attention_mask: np.ndarray) -> np.ndarray:
    """Mirror the reference's index computation with the same jax ops on the
    default backend so the sampled indices match the reference bit-exactly."""
    import jax
    import jax.numpy as jnp

    mask = jnp.asarray(attention_mask)
    leading = jnp.cumprod(mask, axis=1)
    count = leading.sum(axis=1)
    L = jnp.maximum(count - 1, 1).astype(jnp.float32)
    u = jax.random.uniform(jax.random.key(42), (BS, EMB), dtype=jnp.float32)
    idx = jnp.minimum((u * L[:, None]).astype(jnp.int32), SEQ - 1)
    return np.asarray(idx)


# gather order i == output column j; chunk row for (b, j) is
# idx[b, j]*(EMB//ELEM) + j//ELEM within batch b's [SEQ*EMB//ELEM, ELEM] table.
_J = np.arange(NI)
_JDIV = _J // ELEM
_ROWP = _J % 16
_COLP = _J // 16

_MCONST = (np.arange(ELEM)[None, :] == (np.arange(128)[:, None] % ELEM)).astype(np.float32)


def _core_tables(idx_core: np.ndarray) -> np.ndarray:
    """idx_core: [BPC, EMB] int32. Returns [128, BPC*COLS] int16 tables
    (batch b's table at cols [b*COLS, (b+1)*COLS), replicated across the
    8 GPSIMD core partition-groups)."""
    rows = idx_core * (EMB // ELEM) + _JDIV[None, :]           # [BPC, NI]
    g16 = np.zeros((BPC, 16, COLS), dtype=np.int16)
    g16[:, _ROWP, _COLP] = rows.astype(np.int16)
    g = np.tile(g16, (1, 8, 1))                                # [BPC, 128, COLS]
    return np.ascontiguousarray(g.transpose(1, 0, 2).reshape(128, BPC * COLS))


def kernel(output_tokens_from_bert: np.ndarray, attention_mask: np.ndarray) -> np.ndarray:
    from concourse import bass_utils

    tok = np.ascontiguousarray(np.asarray(output_tokens_from_bert, dtype=np.float32))
    idx = _indices(np.asarray(attention_mask))

    in_maps = []
    for c in range(NCORES):
        in_maps.append({
            "tok": tok[c * BPC:(c + 1) * BPC],
            "gidx": _core_tables(idx[c * BPC:(c + 1) * BPC]),
            "mconst": _MCONST,
        })

    nc = _get_module()
    res = bass_utils.run_bass_kernel_spmd(nc, in_maps, core_ids=list(range(NCORES)))
    out = np.concatenate([res.results[c]["out"] for c in range(NCORES)], axis=0)
    return np.ascontiguousarray(out.astype(np.float32))
